# revision 1
# baseline (speedup 1.0000x reference)
"""Trainium2 Bass kernel for the DTI R-GCN (bdd) model, 8 NeuronCores.

Strategy (SPMD, one program, per-core data):
  - dst-shard the graph: core c owns nodes [c*2500, (c+1)*2500); host routes
    each edge to its dst owner and buckets it into (dst-tile, relation) cells,
    tiles of 128 dst nodes.
  - per 128-edge chunk: indirect-DMA gather of x[src] rows (1KB each) onto
    partitions, build a scatter matrix S[e, d] = norm_e * (iota == dstloc_e)
    with one tensor_scalar op, and matmul xg^T @ S to accumulate the
    transposed per-(tile, rel) aggregate aggT[feat, dst] in PSUM.  Pad slots
    use an out-of-bounds src index (descriptor skipped -> no DMA traffic) and
    norm 0.
  - per cell: two [128,128]x[128,128] matmuls with the block-diagonal
    relation weight apply W_r and accumulate msg[d, of] in PSUM across all
    16 relations; the self-loop term x_own @ loop_w joins the same PSUM
    accumulation (x_own^T built with PE transposes).
  - layer output (own 2500 rows) is AllGather'd so every core has the full
    [20000, 256] activations for the next layer's gathers.
  - MLP head is data-parallel over pairs (512 per core) after a final
    AllGather of h2.

The program is built fresh per invocation from the actual inputs (host does
all index preprocessing; trip counts are data-dependent but identical across
cores by padding cells to the max chunk count over cores).
"""
import sys

sys.path.insert(0, "/opt/trn_rl_repo")
import numpy as np

P = 128
NCORES = 8
OOB = np.int32(2**28)


def _preprocess(inputs, ncores=NCORES):
    node_ids = np.asarray(inputs["node_ids"])
    src = np.asarray(inputs["src"])
    dst = np.asarray(inputs["dst"])
    etype = np.asarray(inputs["etype"])
    norm = np.asarray(inputs["norm"]).reshape(-1)
    emb = np.asarray(inputs["emb"], dtype=np.float32)
    drugs = np.asarray(inputs["drugs_index"])
    targets = np.asarray(inputs["targets_index"])

    N = node_ids.shape[0]
    H = emb.shape[1]
    R = int(inputs["w1"].shape[0])
    PAIRS = drugs.shape[0]
    assert N % ncores == 0 and PAIRS % ncores == 0
    NOWN = N // ncores
    TILES = -(-NOWN // P)
    NCELL = (-(-NOWN // (2 * P))) * R
    PPC = PAIRS // ncores
    assert PPC % P == 0
    Q = PPC // P

    TILE2 = 2 * P  # dst nodes per aggregation cell (segments on matmul free dim)
    T2 = -(-NOWN // TILE2)
    owner = dst // NOWN
    d_local = dst - owner * NOWN
    t_of_e = d_local // TILE2
    dstloc_of_e = (d_local % TILE2).astype(np.float32)
    cell_of_e = t_of_e * R + etype

    counts = np.zeros((ncores, NCELL), np.int64)
    for c in range(ncores):
        counts[c] = np.bincount(cell_of_e[owner == c], minlength=NCELL)
    nch = -(-counts.max(axis=0) // P)  # chunks per cell (0 if empty everywhere)
    chunk_start = np.zeros(NCELL, np.int64)
    chunk_start[1:] = np.cumsum(nch)[:-1]
    TC = int(nch.sum())

    srcT = np.full((ncores, P, TC), OOB, np.int32)
    dstlocT = np.zeros((ncores, P, TC), np.float32)
    normT = np.zeros((ncores, P, TC), np.float32)
    for c in range(ncores):
        m = owner == c
        eidx = np.where(m)[0]
        cell = cell_of_e[eidx]
        order = np.argsort(cell, kind="stable")
        eidx = eidx[order]
        cell = cell[order]
        cstart = np.zeros(NCELL, np.int64)
        cstart[1:] = np.cumsum(counts[c])[:-1]
        rank = np.arange(len(eidx)) - cstart[cell]
        col = chunk_start[cell] + rank // P
        part = rank % P
        srcT[c, part, col] = src[eidx]
        dstlocT[c, part, col] = dstloc_of_e[eidx]
        normT[c, part, col] = norm[eidx]

    # host-side embedding lookup: pure data movement, shrinks per-core upload
    # from the full table to the active [N, H] slab
    h0 = emb[node_ids]  # [N, H]
    x0own = np.zeros((ncores, TILES * P, H), np.float32)
    for c in range(ncores):
        x0own[c, :NOWN] = h0[c * NOWN : (c + 1) * NOWN]

    drugsT = drugs.reshape(ncores, Q, P).transpose(0, 2, 1).astype(np.int32).copy()
    targetsT = targets.reshape(ncores, Q, P).transpose(0, 2, 1).astype(np.int32).copy()

    # relation block weights as lhsT [if_local, of_local] per (layer, rel, half)
    B = int(inputs["w1"].shape[1])
    si = H // B
    hb = (P // si)  # blocks per half
    wblk = np.zeros((2, R, 2, P, P), np.float32)
    for l, W in enumerate([inputs["w1"], inputs["w2"]]):
        W = np.asarray(W, np.float32)
        for r in range(R):
            for h in range(2):
                for bb in range(hb):
                    b = hb * h + bb
                    wblk[l, r, h, bb * si : (bb + 1) * si, bb * si : (bb + 1) * si] = W[r, b]
    wblk_in = wblk.transpose(3, 0, 1, 2, 4).reshape(P, 2 * R * 2 * P).copy()

    loopw = np.stack(
        [np.asarray(inputs["loop_w1"], np.float32), np.asarray(inputs["loop_w2"], np.float32)]
    )  # [2, H, H]
    loopw_in = loopw.reshape(2, 2, P, H).transpose(2, 0, 1, 3).reshape(P, 2 * 2 * H).copy()

    bias_in = np.concatenate(
        [
            np.tile(np.asarray(inputs["b1"], np.float32)[None, :], (P, 1)),
            np.tile(np.asarray(inputs["b2"], np.float32)[None, :], (P, 1)),
        ],
        axis=1,
    )  # [P, 2H]

    d2 = 2 * H
    KC = d2 // P  # fc1 contraction chunks
    MC = d2 // P  # fc1 output chunks
    fc1_in = (
        np.asarray(inputs["fc1_W"], np.float32)
        .reshape(KC, P, MC, P)
        .transpose(1, 0, 2, 3)
        .reshape(P, KC * MC * P)
        .copy()
    )
    fc1b_in = np.asarray(inputs["fc1_b"], np.float32).reshape(MC, P).T.copy()
    fc2_in = np.asarray(inputs["fc2_W"], np.float32).reshape(MC, P).T.copy()
    fc2b = float(np.asarray(inputs["fc2_b"]).reshape(-1)[0])

    iota = np.tile(np.arange(2 * P, dtype=np.float32), (P, 1))

    meta = dict(
        N=N, H=H, R=R, NOWN=NOWN, TILES=TILES, T2=T2, NCELL=NCELL, TC=TC, Q=Q,
        KC=KC, MC=MC, nch=nch, chunk_start=chunk_start, fc2b=fc2b,
    )
    shared = dict(
        h0=h0, iota=iota, wblk=wblk_in, loopw=loopw_in, biasbc=bias_in,
        fc1=fc1_in, fc1b=fc1b_in, fc2=fc2_in,
    )
    in_maps = []
    for c in range(ncores):
        m = dict(shared)
        m.update(
            srcT=srcT[c], dstlocT=dstlocT[c], normT=normT[c],
            x0own=x0own[c], drugsT=drugsT[c], targetsT=targetsT[c],
        )
        in_maps.append(m)
    return meta, in_maps


def _build(meta, ncores=NCORES, single=False):
    from concourse import bass, mybir, tile, bacc
    from concourse.masks import make_identity

    N, H, R = meta["N"], meta["H"], meta["R"]
    NOWN, TILES, TC, Q = meta["NOWN"], meta["TILES"], meta["TC"], meta["Q"]
    T2 = meta["T2"]
    KC, MC = meta["KC"], meta["MC"]
    nch, chunk_start = meta["nch"], meta["chunk_start"]
    f32 = mybir.dt.float32
    i32 = mybir.dt.int32

    nc = bacc.Bacc(
        "TRN2", target_bir_lowering=False, debug=False,
        num_devices=(1 if single else ncores),
    )

    h0_t = nc.dram_tensor("h0", [N, H], f32, kind="ExternalInput")
    srcT_t = nc.dram_tensor("srcT", [P, TC], i32, kind="ExternalInput")
    dstlocT_t = nc.dram_tensor("dstlocT", [P, TC], f32, kind="ExternalInput")
    normT_t = nc.dram_tensor("normT", [P, TC], f32, kind="ExternalInput")
    x0own_t = nc.dram_tensor("x0own", [TILES * P, H], f32, kind="ExternalInput")
    drugsT_t = nc.dram_tensor("drugsT", [P, Q], i32, kind="ExternalInput")
    targetsT_t = nc.dram_tensor("targetsT", [P, Q], i32, kind="ExternalInput")
    iota_t = nc.dram_tensor("iota", [P, 2 * P], f32, kind="ExternalInput")
    wblk_t = nc.dram_tensor("wblk", [P, 2 * R * 2 * P], f32, kind="ExternalInput")
    loopw_t = nc.dram_tensor("loopw", [P, 2 * 2 * H], f32, kind="ExternalInput")
    biasbc_t = nc.dram_tensor("biasbc", [P, 2 * H], f32, kind="ExternalInput")
    fc1_t = nc.dram_tensor("fc1", [P, KC * MC * P], f32, kind="ExternalInput")
    fc1b_t = nc.dram_tensor("fc1b", [P, MC], f32, kind="ExternalInput")
    fc2_t = nc.dram_tensor("fc2", [P, MC], f32, kind="ExternalInput")
    out_t = nc.dram_tensor("out", [Q * P, 1], f32, kind="ExternalOutput")

    with tile.TileContext(nc) as tc:
        with (
            tc.tile_pool(name="const", bufs=1) as cp,
            tc.tile_pool(name="work", bufs=10) as wp,
            tc.tile_pool(name="ps", bufs=1, space="PSUM") as pp,
        ):
            # ---- DRAM internals; AllGather is split into piece collectives so
            # each piece can fire as soon as its tiles are stored (overlaps the
            # rest of the layer), then one strided DMA folds it into the
            # node-indexed full table.
            tpp = max(1, -(-TILES // 4))  # tiles per AG piece
            pieces = []  # (row0, nrows)
            for p0 in range(0, TILES, tpp):
                row0 = p0 * P
                nrows = min(NOWN, (p0 + tpp) * P) - row0
                if nrows > 0:
                    pieces.append((row0, nrows))
            h1_own = nc.dram_tensor("h1_own", [TILES * P, H], f32, kind="Internal").ap()
            h1_full = nc.dram_tensor("h1_full", [N, H], f32, kind="Internal").ap()
            h2_full = nc.dram_tensor("h2_full", [N, H], f32, kind="Internal").ap()
            agin = {}
            agout = {}
            for li in (1, 2):
                for pi, (row0, nrows) in enumerate(pieces):
                    agin[(li, pi)] = nc.dram_tensor(
                        f"h{li}_agin{pi}", [nrows, H], f32, kind="Internal"
                    ).ap()
                    agout[(li, pi)] = nc.dram_tensor(
                        f"h{li}_agout{pi}", [ncores * nrows, H], f32,
                        kind="Internal", addr_space="Shared",
                    ).ap()

            # ---- resident constants ----
            srcT = cp.tile([P, TC], i32, name="srcT")
            nc.sync.dma_start(srcT[:], srcT_t.ap()[:])
            dstlocT = cp.tile([P, TC], f32, name="dstlocT")
            nc.sync.dma_start(dstlocT[:], dstlocT_t.ap()[:])
            normT = cp.tile([P, TC], f32, name="normT")
            nc.sync.dma_start(normT[:], normT_t.ap()[:])
            drugsT = cp.tile([P, Q], i32, name="drugsT")
            nc.sync.dma_start(drugsT[:], drugsT_t.ap()[:])
            targetsT = cp.tile([P, Q], i32, name="targetsT")
            nc.sync.dma_start(targetsT[:], targetsT_t.ap()[:])
            iota_sb = cp.tile([P, 2 * P], f32, name="iota_sb")
            nc.sync.dma_start(iota_sb[:], iota_t.ap()[:])
            wblk_sb = cp.tile([P, 2 * R * 2 * P], f32, name="wblk_sb")
            nc.sync.dma_start(wblk_sb[:], wblk_t.ap()[:])
            loopw_sb = cp.tile([P, 2 * 2 * H], f32, name="loopw_sb")
            nc.sync.dma_start(loopw_sb[:], loopw_t.ap()[:])
            biasbc_sb = cp.tile([P, 2 * H], f32, name="biasbc_sb")
            nc.sync.dma_start(biasbc_sb[:], biasbc_t.ap()[:])
            fc1_sb = cp.tile([P, KC * MC * P], f32, name="fc1_sb")
            nc.sync.dma_start(fc1_sb[:], fc1_t.ap()[:])
            fc1b_sb = cp.tile([P, MC], f32, name="fc1b_sb")
            nc.sync.dma_start(fc1b_sb[:], fc1b_t.ap()[:])
            fc2_sb = cp.tile([P, MC], f32, name="fc2_sb")
            nc.sync.dma_start(fc2_sb[:], fc2_t.ap()[:])
            ident = cp.tile([P, P], f32, name="ident")
            make_identity(nc, ident[:])

            def wblk_ap(l, r, h):
                o = ((l * R + r) * 2 + h) * P
                return wblk_sb[:, o : o + P]

            def loopw_ap(l, h):
                o = (l * 2 + h) * H
                return loopw_sb[:, o : o + H]

            # warm the xg pool slots with finite data (OOB-skipped pad rows
            # keep whatever the slot held; must never be NaN/Inf)
            for w in range(10):
                xg = wp.tile([P, H], f32, name="xgwarm", tag="xg")
                nc.sync.dma_start(xg[:], h0_t.ap()[0:P, :])

            def emit_ag_piece(li, pi, h_full):
                row0, nrows = pieces[pi]
                if single:
                    nc.sync.dma_start(
                        h_full[row0 : row0 + nrows, :], agin[(li, pi)][:]
                    )
                    return
                nc.gpsimd.collective_compute(
                    "AllGather", mybir.AluOpType.bypass,
                    replica_groups=[list(range(ncores))],
                    ins=[agin[(li, pi)]], outs=[agout[(li, pi)]],
                )
                src_ap = agout[(li, pi)].rearrange("(c n) h -> c n h", c=ncores)
                dst_ap = h_full.rearrange("(c n) h -> c n h", c=ncores)[
                    :, row0 : row0 + nrows, :
                ]
                nc.sync.dma_start(dst_ap, src_ap)

            def layer(l, src_sb, xsrc_ap, xsrc_rows, xown_ap, out_pad_ap, li, h_full_out, gdt=f32):
                for t2 in range(T2):
                    rels = [r for r in range(R) if nch[t2 * R + r] > 0]
                    subs = [st for st in (2 * t2, 2 * t2 + 1) if st < TILES]
                    msg_ps = {}
                    for si_, st in enumerate(subs):
                        xown_sb = wp.tile([P, H], f32, name="xown", tag="xg")
                        nc.sync.dma_start(xown_sb[:], xown_ap[st * P : (st + 1) * P, :])
                        for h in range(2):
                            tp_ps = pp.tile([P, P], f32, name="tp", tag="agg0", bufs=2)
                            nc.tensor.transpose(
                                tp_ps[:], xown_sb[:, h * P : (h + 1) * P], ident[:]
                            )
                            xT_sb = wp.tile([P, P], f32, name="xT", tag="xT")
                            nc.vector.tensor_copy(xT_sb[:], tp_ps[:])
                            for ho in range(2):
                                if h == 0:
                                    msg_ps[(si_, ho)] = pp.tile(
                                        [P, P], f32, name=f"msg{si_}{ho}",
                                        tag=f"msg{si_}{ho}", bufs=1,
                                    )
                                nc.tensor.matmul(
                                    msg_ps[(si_, ho)][:], lhsT=xT_sb[:],
                                    rhs=loopw_ap(l, h)[:, ho * P : (ho + 1) * P],
                                    start=(h == 0), stop=(h == 1 and not rels),
                                )
                    for ri, r in enumerate(rels):
                        cell = t2 * R + r
                        cs = int(chunk_start[cell])
                        n = int(nch[cell])
                        aggT_ps = [
                            pp.tile([P, 2 * P], f32, name=f"agg{h}", tag=f"agg{h}", bufs=2)
                            for h in range(2)
                        ]
                        for ci in range(n):
                            col = cs + ci
                            xg = wp.tile([P, H], f32, name="xg", tag="xg")
                            nc.gpsimd.indirect_dma_start(
                                out=xg[:], out_offset=None, in_=xsrc_ap,
                                in_offset=bass.IndirectOffsetOnAxis(
                                    ap=src_sb[:, col : col + 1], axis=0
                                ),
                                bounds_check=xsrc_rows - 1, oob_is_err=False,
                            )
                            S = wp.tile([P, 2 * P], f32, name="S", tag="S")
                            nc.vector.tensor_scalar(
                                out=S[:], in0=iota_sb[:],
                                scalar1=dstlocT[:, col : col + 1],
                                scalar2=normT[:, col : col + 1],
                                op0=mybir.AluOpType.is_equal, op1=mybir.AluOpType.mult,
                            )
                            for h in range(2):
                                nc.tensor.matmul(
                                    aggT_ps[h][:], lhsT=xg[:, h * P : (h + 1) * P],
                                    rhs=S[:], start=(ci == 0), stop=(ci == n - 1),
                                )
                        last_rel = ri == len(rels) - 1
                        for h in range(2):
                            aggT_sb = wp.tile(
                                [P, 2 * P], f32, name=f"aggsb{h}", tag=f"aggsb{h}"
                            )
                            if h == 0:
                                nc.vector.tensor_copy(aggT_sb[:], aggT_ps[h][:])
                            else:
                                nc.scalar.copy(aggT_sb[:], aggT_ps[h][:])
                            for si_, st in enumerate(subs):
                                nc.tensor.matmul(
                                    msg_ps[(si_, h)][:],
                                    lhsT=aggT_sb[:, si_ * P : (si_ + 1) * P],
                                    rhs=wblk_ap(l, r, h), start=False, stop=last_rel,
                                )
                    for si_, st in enumerate(subs):
                        out_sb = wp.tile([P, H], f32, name="outsb", tag="xg")
                        for ho in range(2):
                            nc.vector.tensor_tensor(
                                out=out_sb[:, ho * P : (ho + 1) * P],
                                in0=msg_ps[(si_, ho)][:],
                                in1=biasbc_sb[:, l * H + ho * P : l * H + (ho + 1) * P],
                                op=mybir.AluOpType.add,
                            )
                        if out_pad_ap is not None:
                            nc.sync.dma_start(
                                out_pad_ap[st * P : (st + 1) * P, :], out_sb[:]
                            )
                        rows = min(P, NOWN - st * P)
                        pi = st // tpp
                        off = (st - pi * tpp) * P
                        nc.sync.dma_start(
                            agin[(li, pi)][off : off + rows, :], out_sb[:rows, :]
                        )
                        # fire this piece's AllGather as soon as its tiles are
                        # stored so the collective overlaps the rest of the layer
                        if st == TILES - 1 or (st + 1) % tpp == 0:
                            emit_ag_piece(li, pi, h_full_out)

            layer(0, srcT, h0_t.ap()[:], N, x0own_t.ap(), h1_own, 1, h1_full)
            layer(1, srcT, h1_full[:], N, h1_own, None, 2, h2_full)

            # ---- MLP head over this core's Q*P pairs ----
            for q in range(Q):
                xcat = wp.tile([P, 2 * H], f32, name="xcat", tag="xcat")
                nc.gpsimd.indirect_dma_start(
                    out=xcat[:, 0:H], out_offset=None, in_=h2_full[:],
                    in_offset=bass.IndirectOffsetOnAxis(ap=drugsT[:, q : q + 1], axis=0),
                )
                nc.gpsimd.indirect_dma_start(
                    out=xcat[:, H : 2 * H], out_offset=None, in_=h2_full[:],
                    in_offset=bass.IndirectOffsetOnAxis(ap=targetsT[:, q : q + 1], axis=0),
                )
                xcatT = []
                for k in range(KC):
                    tp_ps = pp.tile([P, P], f32, name="tpm", tag="agg0", bufs=2)
                    nc.tensor.transpose(tp_ps[:], xcat[:, k * P : (k + 1) * P], ident[:])
                    xcT = wp.tile([P, P], f32, name=f"xcT{k}", tag=f"xcT{k}")
                    nc.vector.tensor_copy(xcT[:], tp_ps[:])
                    xcatT.append(xcT)
                z_ps = pp.tile([1, P], f32, name="z", tag="agg1", bufs=2)
                for m in range(MC):
                    yT_ps = pp.tile([P, P], f32, name="yT", tag="msg00", bufs=1)
                    for k in range(KC):
                        nc.tensor.matmul(
                            yT_ps[:], lhsT=fc1_sb[:, (k * MC + m) * P : (k * MC + m + 1) * P],
                            rhs=xcatT[k][:], start=(k == 0), stop=(k == KC - 1),
                        )
                    yTr = wp.tile([P, P], f32, name="yTr", tag="S")
                    nc.scalar.activation(
                        yTr[:], yT_ps[:], mybir.ActivationFunctionType.Relu,
                        bias=fc1b_sb[:, m : m + 1], scale=1.0,
                    )
                    nc.tensor.matmul(
                        z_ps[:], lhsT=fc2_sb[:, m : m + 1], rhs=yTr[:],
                        start=(m == 0), stop=(m == MC - 1),
                    )
                zs = wp.tile([1, P], f32, name="zs", tag="zs")
                nc.scalar.activation(
                    zs[:], z_ps[:], mybir.ActivationFunctionType.Sigmoid,
                    bias=meta["fc2b"], scale=1.0,
                )
                nc.sync.dma_start(out_t.ap()[q * P : (q + 1) * P, :], zs[:])
    return nc


_NC_CACHE = []


def kernel(**inputs):
    from concourse import bass_utils

    meta, in_maps = _preprocess(inputs)
    key = (meta["N"], meta["H"], meta["R"], meta["TC"], meta["Q"],
           tuple(int(x) for x in meta["nch"]))
    if _NC_CACHE and _NC_CACHE[0][0] == key:
        nc = _NC_CACHE[0][1]
    else:
        nc = _build(meta)
        nc.compile()
        _NC_CACHE[:] = [(key, nc)]
    res = bass_utils.run_bass_kernel_spmd(nc, in_maps, core_ids=list(range(NCORES)))
    out = np.concatenate([res.results[c]["out"] for c in range(NCORES)], axis=0)
    return out.astype(np.float32)



# revision 11
# speedup vs baseline: 3.2954x; 3.2954x over previous
"""Trainium2 Bass kernel for the DTI R-GCN (bdd) model, 8 NeuronCores.

Strategy (SPMD, one program, per-core data):
  - dst-shard the graph: core c owns nodes [c*2500, (c+1)*2500); host routes
    each edge to its dst owner and buckets it into (dst-tile, relation) cells,
    dst tiles of 256 nodes, edge chunks of 128 (the PE contraction dim).
  - all tensors on the PE path are fp16 (1 cycle/row vs 4 for fp32; PSUM
    accumulation stays fp32), features are gathered as 512B fp16 rows.
  - gathers use the batched int16-index dma_gather (up to 8 chunks = 1024
    rows per instruction) to amortize the SWDGE fixed cost; pad slots point
    at row 0 with norm 0 so they contribute nothing.
  - per 128-edge chunk: scatter matrix S[e, d] = norm_e * (iota == dstloc_e)
    built in one fp16 tensor_scalar (DVE 2x mode), then 2 matmuls accumulate
    the transposed per-(tile, rel) aggregate aggT[if_half, dst] in PSUM.
  - per cell: one combined [P,512] PSUM->SBUF fp16 copy (alternating
    DVE/Act), then 4 relation-block matmuls accumulate msg[dst, of] in PSUM;
    the relation GEMM lags one relation behind the aggregation so the PE
    never waits on the copy.
  - self-loop x^T comes straight from a transpose-mode gather of the node
    table (no PE transposes), accumulated into the same msg PSUM.
  - layer output (own 2500 rows) is AllGather'd piecewise so every core has
    the full [20000, 256] fp16 activations for the next layer's gathers.
  - MLP head is data-parallel over pairs (512 per core): one transpose-mode
    gather delivers xcat^T per 128-pair block, so fc1/fc2 run with weights
    as natural lhsT and no transposes.
"""
import sys

sys.path.insert(0, "/opt/trn_rl_repo")
import numpy as np

P = 128
NCORES = 8
TILE2 = 2 * P  # dst nodes per aggregation cell
GK = 8  # max chunks per dma_gather


def _wrap16(arr):
    """int sequence [n] (n % 16 == 0) -> wrapped idx layout [128, n//16]:
    idx i at (i % 16, i // 16), replicated across the 8 partition groups."""
    n = arr.shape[0]
    blk = arr.reshape(n // 16, 16).T
    return np.tile(blk, (8, 1)).astype(np.int16)


def _preprocess(inputs, ncores=NCORES):
    node_ids = np.asarray(inputs["node_ids"])
    src = np.asarray(inputs["src"])
    dst = np.asarray(inputs["dst"])
    etype = np.asarray(inputs["etype"])
    norm = np.asarray(inputs["norm"]).reshape(-1)
    emb = np.asarray(inputs["emb"], dtype=np.float32)
    drugs = np.asarray(inputs["drugs_index"])
    targets = np.asarray(inputs["targets_index"])

    N = node_ids.shape[0]
    H = emb.shape[1]
    R = int(inputs["w1"].shape[0])
    PAIRS = drugs.shape[0]
    assert N % ncores == 0 and PAIRS % ncores == 0
    NOWN = N // ncores
    TILES = -(-NOWN // P)
    PPC = PAIRS // ncores
    assert PPC % P == 0
    Q = PPC // P

    T2 = -(-NOWN // TILE2)
    NCELL = T2 * R
    owner = dst // NOWN
    d_local = dst - owner * NOWN
    t_of_e = d_local // TILE2
    dstloc_of_e = (d_local % TILE2).astype(np.float32)
    cell_of_e = t_of_e * R + etype

    counts = np.zeros((ncores, NCELL), np.int64)
    for c in range(ncores):
        counts[c] = np.bincount(cell_of_e[owner == c], minlength=NCELL)
    nch = -(-counts.max(axis=0) // P)  # chunks per cell (0 if empty everywhere)
    chunk_start = np.zeros(NCELL, np.int64)
    chunk_start[1:] = np.cumsum(nch)[:-1]
    TC = int(nch.sum())

    srcT = np.zeros((ncores, P, TC), np.int16)  # pad slots -> row 0
    dstlocT = np.zeros((ncores, P, TC), np.float32)
    normT = np.zeros((ncores, P, TC), np.float32)
    for c in range(ncores):
        m = owner == c
        eidx = np.where(m)[0]
        cell = cell_of_e[eidx]
        order = np.argsort(cell, kind="stable")
        eidx = eidx[order]
        cell = cell[order]
        cstart = np.zeros(NCELL, np.int64)
        cstart[1:] = np.cumsum(counts[c])[:-1]
        rank = np.arange(len(eidx)) - cstart[cell]
        col = chunk_start[cell] + rank // P
        part = rank % P
        srcT[c, part, col] = src[eidx]
        dstlocT[c, part, col] = dstloc_of_e[eidx]
        normT[c, part, col] = norm[eidx]

    # gather groups: per t2, consecutive chunk columns in groups of <= GK
    groups = []  # (col0, k, goff)
    gmap = {}  # chunk col -> (gid, offset within group)
    goff = 0
    for t2 in range(T2):
        cs = int(chunk_start[t2 * R])
        ce = cs + int(nch[t2 * R : (t2 + 1) * R].sum())
        for c0 in range(cs, ce, GK):
            k = min(GK, ce - c0)
            gid = len(groups)
            groups.append((c0, k, goff))
            for j in range(k):
                gmap[c0 + j] = (gid, j)
            goff += k * 8
    GW = goff

    idx16 = np.zeros((ncores, P, GW), np.int16)
    for c in range(ncores):
        for c0, k, off in groups:
            vals = srcT[c, :, c0 : c0 + k]  # [P, k]
            arr = vals.T.reshape(k * P)
            idx16[c, :, off : off + k * 8] = _wrap16(arr)

    # self-loop transpose-gather indices: per t2, own rows (pad -> own row 0)
    ownidx16 = np.zeros((ncores, P, T2 * 16), np.int16)
    for c in range(ncores):
        for t2 in range(T2):
            j = t2 * TILE2 + np.arange(TILE2)
            j = np.where(j < NOWN, j, 0) + c * NOWN
            ownidx16[c, :, t2 * 16 : (t2 + 1) * 16] = _wrap16(j)

    # head transpose-gather indices: per q, [drugs block | targets block]
    pairsidx16 = np.zeros((ncores, P, Q * 16), np.int16)
    for c in range(ncores):
        d_c = drugs[c * PPC : (c + 1) * PPC]
        t_c = targets[c * PPC : (c + 1) * PPC]
        for q in range(Q):
            arr = np.concatenate([d_c[q * P : (q + 1) * P], t_c[q * P : (q + 1) * P]])
            pairsidx16[c, :, q * 16 : (q + 1) * 16] = _wrap16(arr)

    # host-side embedding lookup, fp16 node table
    h0 = emb[node_ids].astype(np.float16)  # [N, H]

    # relation block weights as rhs [if_local, of_local] per (layer, rel, half)
    B = int(inputs["w1"].shape[1])
    si = H // B
    hb = P // si  # blocks per half
    wblk = np.zeros((2, R, 2, P, P), np.float16)
    for l, W in enumerate([inputs["w1"], inputs["w2"]]):
        W = np.asarray(W, np.float32)
        for r in range(R):
            for h in range(2):
                for bb in range(hb):
                    b = hb * h + bb
                    wblk[l, r, h, bb * si : (bb + 1) * si, bb * si : (bb + 1) * si] = W[r, b]
    wblk_in = wblk.transpose(3, 0, 1, 2, 4).reshape(P, 2 * R * 2 * P).copy()

    # self-loop weights as lhsT-compatible rhs [if_half rows, of=H] per (l, h)
    loopw = np.stack(
        [np.asarray(inputs["loop_w1"], np.float32), np.asarray(inputs["loop_w2"], np.float32)]
    ).astype(np.float16)  # [2, H, H]
    loopw_in = loopw.reshape(2, 2, P, H).transpose(2, 0, 1, 3).reshape(P, 2 * 2 * H).copy()

    bias_in = np.concatenate(
        [
            np.tile(np.asarray(inputs["b1"], np.float32)[None, :], (P, 1)),
            np.tile(np.asarray(inputs["b2"], np.float32)[None, :], (P, 1)),
        ],
        axis=1,
    )  # [P, 2H] f32

    d2 = 2 * H
    KC = d2 // P
    MC = d2 // P
    # fc1 blocks as natural lhsT [if128, of128] per (ifc, ofc)
    fc1_in = (
        np.asarray(inputs["fc1_W"], np.float16)
        .reshape(KC, P, MC, P)
        .transpose(1, 0, 2, 3)
        .reshape(P, KC * MC * P)
        .copy()
    )
    fc1b_in = np.asarray(inputs["fc1_b"], np.float32).reshape(MC, P).T.copy()
    fc2_in = np.asarray(inputs["fc2_W"], np.float16).reshape(MC, P).T.copy()
    fc2b = float(np.asarray(inputs["fc2_b"]).reshape(-1)[0])

    iota = np.tile(np.arange(TILE2, dtype=np.float16), (P, 1))

    meta = dict(
        N=N, H=H, R=R, NOWN=NOWN, TILES=TILES, T2=T2, NCELL=NCELL, TC=TC, Q=Q,
        KC=KC, MC=MC, GW=GW, nch=nch, chunk_start=chunk_start, groups=groups,
        gmap=gmap, fc2b=fc2b,
    )
    shared = dict(
        h0=h0, iota=iota, wblk=wblk_in, loopw=loopw_in, biasbc=bias_in,
        fc1=fc1_in, fc1b=fc1b_in, fc2=fc2_in,
    )
    in_maps = []
    for c in range(ncores):
        m = dict(shared)
        m.update(
            idx16=idx16[c], dstlocT=dstlocT[c], normT=normT[c],
            ownidx16=ownidx16[c], pairsidx16=pairsidx16[c],
        )
        in_maps.append(m)
    return meta, in_maps


def _build(meta, ncores=NCORES, single=False, dump=False):
    from concourse import bass, mybir, tile, bacc

    N, H, R = meta["N"], meta["H"], meta["R"]
    NOWN, TILES, TC, Q = meta["NOWN"], meta["TILES"], meta["TC"], meta["Q"]
    T2 = meta["T2"]
    KC, MC, GW = meta["KC"], meta["MC"], meta["GW"]
    nch, chunk_start = meta["nch"], meta["chunk_start"]
    groups, gmap = meta["groups"], meta["gmap"]
    f32 = mybir.dt.float32
    f16 = mybir.dt.float16
    i16 = mybir.dt.int16

    nc = bacc.Bacc(
        "TRN2", target_bir_lowering=False, debug=False,
        num_devices=(1 if single else ncores),
        dynamic_dma_scratch_size=65536,
    )

    h0_t = nc.dram_tensor("h0", [N, H], f16, kind="ExternalInput")
    idx16_t = nc.dram_tensor("idx16", [P, GW], i16, kind="ExternalInput")
    dstlocT_t = nc.dram_tensor("dstlocT", [P, TC], f32, kind="ExternalInput")
    normT_t = nc.dram_tensor("normT", [P, TC], f32, kind="ExternalInput")
    ownidx_t = nc.dram_tensor("ownidx16", [P, T2 * 16], i16, kind="ExternalInput")
    pairsidx_t = nc.dram_tensor("pairsidx16", [P, Q * 16], i16, kind="ExternalInput")
    iota_t = nc.dram_tensor("iota", [P, TILE2], f16, kind="ExternalInput")
    wblk_t = nc.dram_tensor("wblk", [P, 2 * R * 2 * P], f16, kind="ExternalInput")
    loopw_t = nc.dram_tensor("loopw", [P, 2 * 2 * H], f16, kind="ExternalInput")
    biasbc_t = nc.dram_tensor("biasbc", [P, 2 * H], f32, kind="ExternalInput")
    fc1_t = nc.dram_tensor("fc1", [P, KC * MC * P], f16, kind="ExternalInput")
    fc1b_t = nc.dram_tensor("fc1b", [P, MC], f32, kind="ExternalInput")
    fc2_t = nc.dram_tensor("fc2", [P, MC], f16, kind="ExternalInput")
    out_t = nc.dram_tensor("out", [Q * P, 1], f32, kind="ExternalOutput")

    with tile.TileContext(nc) as tc:
        with (
            tc.tile_pool(name="const", bufs=1) as cp,
            tc.tile_pool(name="work", bufs=10) as wp,
            tc.tile_pool(name="ps", bufs=1, space="PSUM") as pp,
        ):
            # ---- DRAM internals; AllGather split into piece collectives ----
            tpp = max(1, -(-TILES // 4))  # 128-row tiles per AG piece
            pieces = []  # (row0, nrows)
            for p0 in range(0, TILES, tpp):
                row0 = p0 * P
                nrows = min(NOWN, (p0 + tpp) * P) - row0
                if nrows > 0:
                    pieces.append((row0, nrows))
            h1_full = nc.dram_tensor("h1_full", [N, H], f16, kind="Internal").ap()
            h2_full = nc.dram_tensor("h2_full", [N, H], f16, kind="Internal").ap()
            agin = {}
            agout = {}
            for li in (1, 2):
                for pi, (row0, nrows) in enumerate(pieces):
                    agin[(li, pi)] = nc.dram_tensor(
                        f"h{li}_agin{pi}", [nrows, H], f16, kind="Internal"
                    ).ap()
                    agout[(li, pi)] = nc.dram_tensor(
                        f"h{li}_agout{pi}", [ncores * nrows, H], f16,
                        kind="Internal", addr_space="Shared",
                    ).ap()

            # ---- resident constants ----
            idx16 = cp.tile([P, GW], i16, name="idx16")
            nc.sync.dma_start(idx16[:], idx16_t.ap()[:])
            dstlocT = cp.tile([P, TC], f32, name="dstlocT")
            nc.sync.dma_start(dstlocT[:], dstlocT_t.ap()[:])
            normT = cp.tile([P, TC], f32, name="normT")
            nc.sync.dma_start(normT[:], normT_t.ap()[:])
            ownidx = cp.tile([P, T2 * 16], i16, name="ownidx")
            nc.sync.dma_start(ownidx[:], ownidx_t.ap()[:])
            pairsidx = cp.tile([P, Q * 16], i16, name="pairsidx")
            nc.sync.dma_start(pairsidx[:], pairsidx_t.ap()[:])
            iota_sb = cp.tile([P, TILE2], f16, name="iota_sb")
            nc.sync.dma_start(iota_sb[:], iota_t.ap()[:])
            wblk_sb = cp.tile([P, 2 * R * 2 * P], f16, name="wblk_sb")
            nc.sync.dma_start(wblk_sb[:], wblk_t.ap()[:])
            loopw_sb = cp.tile([P, 2 * 2 * H], f16, name="loopw_sb")
            nc.sync.dma_start(loopw_sb[:], loopw_t.ap()[:])
            biasbc_sb = cp.tile([P, 2 * H], f32, name="biasbc_sb")
            nc.sync.dma_start(biasbc_sb[:], biasbc_t.ap()[:])
            fc1_sb = cp.tile([P, KC * MC * P], f16, name="fc1_sb")
            nc.sync.dma_start(fc1_sb[:], fc1_t.ap()[:])
            fc1b_sb = cp.tile([P, MC], f32, name="fc1b_sb")
            nc.sync.dma_start(fc1b_sb[:], fc1b_t.ap()[:])
            fc2_sb = cp.tile([P, MC], f16, name="fc2_sb")
            nc.sync.dma_start(fc2_sb[:], fc2_t.ap()[:])

            def wblk_ap(l, r, h):
                o = ((l * R + r) * 2 + h) * P
                return wblk_sb[:, o : o + P]

            def loopw_ap(l, h):
                o = (l * 2 + h) * H
                return loopw_sb[:, o : o + H]

            def emit_ag_piece(li, pi, h_full):
                row0, nrows = pieces[pi]
                if single:
                    nc.sync.dma_start(
                        h_full[row0 : row0 + nrows, :], agin[(li, pi)][:]
                    )
                    return
                nc.gpsimd.collective_compute(
                    "AllGather", mybir.AluOpType.bypass,
                    replica_groups=[list(range(ncores))],
                    ins=[agin[(li, pi)]], outs=[agout[(li, pi)]],
                )
                src_ap = agout[(li, pi)].rearrange("(c n) h -> c n h", c=ncores)
                dst_ap = h_full.rearrange("(c n) h -> c n h", c=ncores)[
                    :, row0 : row0 + nrows, :
                ]
                nc.sync.dma_start(dst_ap, src_ap)

            copy_flip = [0]

            def layer(l, xsrc_ap, li, h_full_out):
                for t2 in range(T2):
                    # self-loop x^T via transpose gather: [if%128, if//128, dst]
                    xT = wp.tile([P, 2 * TILE2], f16, name="xT", tag="xT", bufs=3)
                    nc.gpsimd.dma_gather(
                        out_ap=xT[:].rearrange("p (j d) -> p j d", d=TILE2),
                        in_ap=xsrc_ap,
                        idxs_ap=ownidx[:, t2 * 16 : (t2 + 1) * 16],
                        num_idxs=TILE2, num_idxs_reg=TILE2, elem_size=H,
                        transpose=True,
                    )
                    subs = [st for st in (2 * t2, 2 * t2 + 1) if st < TILES]
                    # issue this t2's edge gathers
                    t2_groups = [
                        g for g in groups
                        if chunk_start[t2 * R] <= g[0]
                        and g[0] < chunk_start[t2 * R] + nch[t2 * R : (t2 + 1) * R].sum()
                    ]
                    gtiles = {}
                    for c0, k, off in t2_groups:
                        xg = wp.tile([P, GK * H], f16, name="xg", tag="xg", bufs=14)
                        nc.gpsimd.dma_gather(
                            out_ap=xg[:, : k * H].rearrange("p (k h) -> p k h", h=H),
                            in_ap=xsrc_ap,
                            idxs_ap=idx16[:, off : off + k * 8],
                            num_idxs=k * P, num_idxs_reg=k * P, elem_size=H,
                        )
                        gtiles[c0] = (xg, c0)

                    rels = [r for r in range(R) if nch[t2 * R + r] > 0]
                    msg = {}
                    for si_, st in enumerate(subs):
                        msg[si_] = pp.tile(
                            [P, TILE2], f32, name=f"msg{si_}", tag=f"msg{si_}", bufs=2
                        )
                        for j in range(2):
                            nc.tensor.matmul(
                                msg[si_][:],
                                lhsT=xT[:, j * TILE2 + si_ * P : j * TILE2 + si_ * P + P],
                                rhs=loopw_ap(l, j),
                                start=(j == 0), stop=(j == 1 and not rels),
                            )
                    pending = None  # (aggsb, r) awaiting relation GEMM
                    for ri, r in enumerate(rels):
                        cell = t2 * R + r
                        cs = int(chunk_start[cell])
                        n = int(nch[cell])
                        aggT = [
                            pp.tile([P, TILE2], f32, name=f"aggT{h}", tag=f"aggT{h}", bufs=2)
                            for h in range(2)
                        ]
                        for ci in range(n):
                            col = cs + ci
                            gid, joff = gmap[col]
                            xg, gc0 = gtiles[groups[gid][0]]
                            S = wp.tile([P, TILE2], f16, name="S", tag="S", bufs=6)
                            nc.vector.tensor_scalar(
                                out=S[:], in0=iota_sb[:],
                                scalar1=dstlocT[:, col : col + 1],
                                scalar2=normT[:, col : col + 1],
                                op0=mybir.AluOpType.is_equal, op1=mybir.AluOpType.mult,
                            )
                            for h in range(2):
                                nc.tensor.matmul(
                                    aggT[h][:],
                                    lhsT=xg[:, joff * H + h * P : joff * H + h * P + P],
                                    rhs=S[:], start=(ci == 0), stop=(ci == n - 1),
                                )
                        aggsb = wp.tile([P, 2 * TILE2], f16, name="aggsb", tag="aggsb", bufs=4)
                        if copy_flip[0] % 2 == 0:
                            for h in range(2):
                                nc.vector.tensor_copy(
                                    aggsb[:, h * TILE2 : (h + 1) * TILE2], aggT[h][:]
                                )
                        else:
                            for h in range(2):
                                nc.scalar.copy(
                                    aggsb[:, h * TILE2 : (h + 1) * TILE2], aggT[h][:]
                                )
                        copy_flip[0] += 1

                        if pending is not None:
                            psb, pr = pending
                            for si_, st in enumerate(subs):
                                for h in range(2):
                                    nc.tensor.matmul(
                                        msg[si_][:, h * P : (h + 1) * P],
                                        lhsT=psb[:, h * TILE2 + si_ * P : h * TILE2 + si_ * P + P],
                                        rhs=wblk_ap(l, pr, h), start=False, stop=False,
                                    )
                        pending = (aggsb, r)

                    # drain: last relation GEMM closes the accumulation
                    if pending is not None:
                        psb, pr = pending
                        for si_, st in enumerate(subs):
                            for h in range(2):
                                nc.tensor.matmul(
                                    msg[si_][:, h * P : (h + 1) * P],
                                    lhsT=psb[:, h * TILE2 + si_ * P : h * TILE2 + si_ * P + P],
                                    rhs=wblk_ap(l, pr, h), start=False, stop=True,
                                )

                    for si_, st in enumerate(subs):
                        out_sb = wp.tile([P, H], f16, name="outsb", tag="outsb", bufs=4)
                        nc.vector.tensor_tensor(
                            out=out_sb[:], in0=msg[si_][:],
                            in1=biasbc_sb[:, l * H : (l + 1) * H],
                            op=mybir.AluOpType.add,
                        )
                        rows = min(P, NOWN - st * P)
                        pi = st // tpp
                        off = (st - pi * tpp) * P
                        nc.sync.dma_start(
                            agin[(li, pi)][off : off + rows, :], out_sb[:rows, :]
                        )
                        if st == TILES - 1 or (st + 1) % tpp == 0:
                            emit_ag_piece(li, pi, h_full_out)

            layer(0, h0_t.ap()[:], 1, h1_full)
            layer(1, h1_full[:], 2, h2_full)
            if dump:
                h1d = nc.dram_tensor("h1dump", [N, H], f16, kind="ExternalOutput")
                nc.sync.dma_start(h1d.ap()[:], h1_full[:])
                h2d = nc.dram_tensor("h2dump", [N, H], f16, kind="ExternalOutput")
                nc.sync.dma_start(h2d.ap()[:], h2_full[:])

            # ---- MLP head over this core's Q*P pairs (transposed layout) ----
            for q in range(Q):
                xpT = wp.tile([P, 2 * 2 * P], f16, name="xpT", tag="xpT", bufs=2)
                nc.gpsimd.dma_gather(
                    out_ap=xpT[:].rearrange("p (j d) -> p j d", d=2 * P),
                    in_ap=h2_full[:],
                    idxs_ap=pairsidx[:, q * 16 : (q + 1) * 16],
                    num_idxs=2 * P, num_idxs_reg=2 * P, elem_size=H,
                    transpose=True,
                )

                # xcat chunk ifc -> slice of xpT: drug (j, 0:128), target (j, 128:256)
                def xc(ifc):
                    j = ifc % 2
                    half = 0 if ifc < 2 else P
                    return xpT[:, j * 2 * P + half : j * 2 * P + half + P]

                yTr = wp.tile([P, MC * P], f16, name="yTr", tag="yTr", bufs=2)
                z = pp.tile([P, TILE2], f32, name="z", tag="msg0", bufs=2)
                for m in range(MC):
                    yT = pp.tile([P, TILE2], f32, name="yT", tag=f"aggT{m % 2}", bufs=2)
                    for k in range(KC):
                        nc.tensor.matmul(
                            yT[:, 0:P],
                            lhsT=fc1_sb[:, (k * MC + m) * P : (k * MC + m + 1) * P],
                            rhs=xc(k), start=(k == 0), stop=(k == KC - 1),
                        )
                    nc.scalar.activation(
                        yTr[:, m * P : (m + 1) * P], yT[:, 0:P],
                        mybir.ActivationFunctionType.Relu,
                        bias=fc1b_sb[:, m : m + 1], scale=1.0,
                    )
                    nc.tensor.matmul(
                        z[0:1, 0:P], lhsT=fc2_sb[:, m : m + 1],
                        rhs=yTr[:, m * P : (m + 1) * P],
                        start=(m == 0), stop=(m == MC - 1),
                    )
                zs = wp.tile([1, P], f32, name="zs", tag="zs", bufs=2)
                nc.scalar.activation(
                    zs[:], z[0:1, 0:P], mybir.ActivationFunctionType.Sigmoid,
                    bias=meta["fc2b"], scale=1.0,
                )
                nc.sync.dma_start(out_t.ap()[q * P : (q + 1) * P, :], zs[:])
    return nc


_NC_CACHE = []


def kernel(**inputs):
    from concourse import bass_utils

    meta, in_maps = _preprocess(inputs)
    key = (meta["N"], meta["H"], meta["R"], meta["TC"], meta["Q"],
           tuple(int(x) for x in meta["nch"]))
    if _NC_CACHE and _NC_CACHE[0][0] == key:
        nc = _NC_CACHE[0][1]
    else:
        nc = _build(meta)
        nc.compile()
        _NC_CACHE[:] = [(key, nc)]
    res = bass_utils.run_bass_kernel_spmd(nc, in_maps, core_ids=list(range(NCORES)))
    out = np.concatenate([res.results[c]["out"] for c in range(NCORES)], axis=0)
    return out.astype(np.float32)


# revision 23
# speedup vs baseline: 4.1514x; 1.2598x over previous
"""Trainium2 Bass kernel for the DTI R-GCN (bdd) model, 8 NeuronCores.

Strategy (SPMD, one program, per-core data):
  - dst-shard the graph: core c owns nodes [c*2500, (c+1)*2500); host routes
    each edge to its dst owner and buckets it into (dst-tile, relation) cells,
    dst tiles of 256 nodes, edge chunks of 128 (the PE contraction dim).
  - all tensors on the PE path are fp16 (1 cycle/row vs 4 for fp32; PSUM
    accumulation stays fp32), features are gathered as 512B fp16 rows.
  - gathers use the batched int16-index dma_gather (up to 8 chunks = 1024
    rows per instruction) to amortize the SWDGE fixed cost; pad slots point
    at row 0 with norm 0 so they contribute nothing.
  - per 128-edge chunk: scatter matrix S[e, d] = norm_e * (iota == dstloc_e)
    built in one fp16 tensor_scalar (DVE 2x mode), then 2 matmuls accumulate
    the transposed per-(tile, rel) aggregate aggT[if_half, dst] in PSUM.
  - per cell: one combined [P,512] PSUM->SBUF fp16 copy (alternating
    DVE/Act), then 4 relation-block matmuls accumulate msg[dst, of] in PSUM;
    the relation GEMM lags one relation behind the aggregation so the PE
    never waits on the copy.
  - self-loop x^T comes straight from a transpose-mode gather of the node
    table (no PE transposes), accumulated into the same msg PSUM.
  - layer output (own 2500 rows) is AllGather'd piecewise so every core has
    the full [20000, 256] fp16 activations for the next layer's gathers.
  - MLP head is data-parallel over pairs (512 per core): one transpose-mode
    gather delivers xcat^T per 128-pair block, so fc1/fc2 run with weights
    as natural lhsT and no transposes.
"""
import sys

sys.path.insert(0, "/opt/trn_rl_repo")
import numpy as np

P = 128
NCORES = 8
TILE2 = 2 * P  # dst nodes per aggregation cell
GK = 8  # max chunks per dma_gather


def _wrap16(arr):
    """int sequence [n] (n % 16 == 0) -> wrapped idx layout [128, n//16]:
    idx i at (i % 16, i // 16), replicated across the 8 partition groups."""
    n = arr.shape[0]
    blk = arr.reshape(n // 16, 16).T
    return np.tile(blk, (8, 1)).astype(np.int16)


def _preprocess(inputs, ncores=NCORES):
    node_ids = np.asarray(inputs["node_ids"])
    src = np.asarray(inputs["src"])
    dst = np.asarray(inputs["dst"])
    etype = np.asarray(inputs["etype"])
    norm = np.asarray(inputs["norm"]).reshape(-1)
    emb = np.asarray(inputs["emb"], dtype=np.float32)
    drugs = np.asarray(inputs["drugs_index"])
    targets = np.asarray(inputs["targets_index"])

    N = node_ids.shape[0]
    H = emb.shape[1]
    R = int(inputs["w1"].shape[0])
    PAIRS = drugs.shape[0]
    assert N % ncores == 0 and PAIRS % ncores == 0
    NOWN = N // ncores
    TILES = -(-NOWN // P)
    PPC = PAIRS // ncores
    assert PPC % P == 0
    Q = PPC // P

    T2 = -(-NOWN // TILE2)
    NCELL = T2 * R
    owner = dst // NOWN
    d_local = dst - owner * NOWN
    t_of_e = d_local // TILE2
    dstloc_of_e = (d_local % TILE2).astype(np.float32)
    cell_of_e = t_of_e * R + etype

    # per-(cell, dstloc) max-over-cores counts -> disjoint dst-range chunks:
    # each chunk owns a dst range [a, b) of its cell with max-core count <= P,
    # so each dst column is scatter-written exactly once per cell.
    fine = np.zeros((ncores, NCELL * TILE2), np.int64)
    fine_idx = cell_of_e * TILE2 + (d_local % TILE2)
    for c in range(ncores):
        fine[c] = np.bincount(fine_idx[owner == c], minlength=NCELL * TILE2)
    finec = fine.reshape(ncores, NCELL, TILE2)
    counts = finec.sum(axis=2)
    nch = np.zeros(NCELL, np.int64)
    rng_lo = []  # per chunk col: range start (within cell tile)
    rng_w = []   # per chunk col: range width
    for cell in range(NCELL):
        if counts[:, cell].max() == 0:
            continue
        a = 0
        run = np.zeros(ncores, np.int64)
        n = 0
        for d in range(TILE2):
            cnt = finec[:, cell, d]
            assert cnt.max() <= P, "single dst exceeds chunk capacity"
            if (run + cnt).max() > P:
                rng_lo.append(a)
                rng_w.append(d - a)
                a = d
                run[:] = 0
                n += 1
            run += cnt
        rng_lo.append(a)
        rng_w.append(TILE2 - a)
        nch[cell] = n + 1
    chunk_start = np.zeros(NCELL, np.int64)
    chunk_start[1:] = np.cumsum(nch)[:-1]
    TC = int(nch.sum())
    rng_lo = np.asarray(rng_lo, np.int64)
    rng_w = np.asarray(rng_w, np.int64)

    srcT = np.zeros((ncores, P, TC), np.int16)  # pad slots -> row 0
    dstlocT = np.zeros((ncores, P, TC), np.float32)
    normT = np.zeros((ncores, P, TC), np.float32)
    dl256 = d_local % TILE2
    for c in range(ncores):
        m = owner == c
        eidx = np.where(m)[0]
        # order edges by (cell, dstloc) then assign to the chunk owning the range
        order = np.lexsort((dl256[eidx], cell_of_e[eidx]))
        eidx = eidx[order]
        cell = cell_of_e[eidx]
        dl = dl256[eidx]
        # chunk col for each edge: chunk_start[cell] + searchsorted within ranges
        col = np.empty(len(eidx), np.int64)
        for cc in np.unique(cell):
            mm = cell == cc
            cs = int(chunk_start[cc])
            ne = int(nch[cc])
            lo = rng_lo[cs : cs + ne]
            col[mm] = cs + np.searchsorted(lo, dl[mm], side="right") - 1
        # slot within chunk = running rank per chunk col
        uniq, inv, cnts = np.unique(col, return_inverse=True, return_counts=True)
        starts = np.zeros(len(uniq), np.int64)
        starts[1:] = np.cumsum(cnts)[:-1]
        part = np.arange(len(eidx)) - starts[inv]
        assert part.max() < P
        srcT[c, part, col] = src[eidx]
        dstlocT[c, part, col] = dstloc_of_e[eidx]
        normT[c, part, col] = norm[eidx]

    # gather groups: per t2, consecutive chunk columns in groups of <= GK
    groups = []  # (col0, k, goff)
    gmap = {}  # chunk col -> (gid, offset within group)
    goff = 0
    for t2 in range(T2):
        cs = int(chunk_start[t2 * R])
        ce = cs + int(nch[t2 * R : (t2 + 1) * R].sum())
        for c0 in range(cs, ce, GK):
            k = min(GK, ce - c0)
            gid = len(groups)
            groups.append((c0, k, goff))
            for j in range(k):
                gmap[c0 + j] = (gid, j)
            goff += k * 8
    GW = goff

    idx16 = np.zeros((ncores, P, GW), np.int16)
    for c in range(ncores):
        for c0, k, off in groups:
            vals = srcT[c, :, c0 : c0 + k]  # [P, k]
            arr = vals.T.reshape(k * P)
            idx16[c, :, off : off + k * 8] = _wrap16(arr)

    # self-loop transpose-gather indices: per t2, own rows (pad -> own row 0)
    ownidx16 = np.zeros((ncores, P, T2 * 16), np.int16)
    for c in range(ncores):
        for t2 in range(T2):
            j = t2 * TILE2 + np.arange(TILE2)
            j = np.where(j < NOWN, j, 0) + c * NOWN
            ownidx16[c, :, t2 * 16 : (t2 + 1) * 16] = _wrap16(j)

    # head transpose-gather indices: per q, [drugs block | targets block]
    pairsidx16 = np.zeros((ncores, P, Q * 16), np.int16)
    for c in range(ncores):
        d_c = drugs[c * PPC : (c + 1) * PPC]
        t_c = targets[c * PPC : (c + 1) * PPC]
        for q in range(Q):
            arr = np.concatenate([d_c[q * P : (q + 1) * P], t_c[q * P : (q + 1) * P]])
            pairsidx16[c, :, q * 16 : (q + 1) * 16] = _wrap16(arr)

    # host-side embedding lookup, fp16 node table
    h0 = emb[node_ids].astype(np.float16)  # [N, H]

    # relation block weights as rhs [if_local, of_local] per (layer, rel, half)
    B = int(inputs["w1"].shape[1])
    si = H // B
    hb = P // si  # blocks per half
    wblk = np.zeros((2, R, 2, P, P), np.float16)
    for l, W in enumerate([inputs["w1"], inputs["w2"]]):
        W = np.asarray(W, np.float32)
        for r in range(R):
            for h in range(2):
                for bb in range(hb):
                    b = hb * h + bb
                    wblk[l, r, h, bb * si : (bb + 1) * si, bb * si : (bb + 1) * si] = W[r, b]
    wblk_in = wblk.transpose(3, 0, 1, 2, 4).reshape(P, 2 * R * 2 * P).copy()

    # self-loop weights as lhsT-compatible rhs [if_half rows, of=H] per (l, h)
    loopw = np.stack(
        [np.asarray(inputs["loop_w1"], np.float32), np.asarray(inputs["loop_w2"], np.float32)]
    ).astype(np.float16)  # [2, H, H]
    loopw_in = loopw.reshape(2, 2, P, H).transpose(2, 0, 1, 3).reshape(P, 2 * 2 * H).copy()

    bias_in = np.concatenate(
        [
            np.tile(np.asarray(inputs["b1"], np.float32)[None, :], (P, 1)),
            np.tile(np.asarray(inputs["b2"], np.float32)[None, :], (P, 1)),
        ],
        axis=1,
    )  # [P, 2H] f32

    d2 = 2 * H
    KC = d2 // P
    MC = d2 // P
    # fc1 blocks as natural lhsT [if128, of128] per (ifc, ofc)
    fc1_in = (
        np.asarray(inputs["fc1_W"], np.float16)
        .reshape(KC, P, MC, P)
        .transpose(1, 0, 2, 3)
        .reshape(P, KC * MC * P)
        .copy()
    )
    fc1b_in = np.asarray(inputs["fc1_b"], np.float32).reshape(MC, P).T.copy()
    fc2_in = np.asarray(inputs["fc2_W"], np.float16).reshape(MC, P).T.copy()
    fc2b = float(np.asarray(inputs["fc2_b"]).reshape(-1)[0])

    iota = np.tile(np.arange(TILE2, dtype=np.float16), (P, 1))

    meta = dict(
        N=N, H=H, R=R, NOWN=NOWN, TILES=TILES, T2=T2, NCELL=NCELL, TC=TC, Q=Q,
        KC=KC, MC=MC, GW=GW, nch=nch, chunk_start=chunk_start, groups=groups,
        gmap=gmap, rng_lo=rng_lo, rng_w=rng_w, fc2b=fc2b,
    )
    shared = dict(
        h0=h0, iota=iota, wblk=wblk_in, loopw=loopw_in, biasbc=bias_in,
        fc1=fc1_in, fc1b=fc1b_in, fc2=fc2_in,
    )
    in_maps = []
    for c in range(ncores):
        m = dict(shared)
        m.update(
            idx16=idx16[c], dstlocT=dstlocT[c], normT=normT[c],
            ownidx16=ownidx16[c], pairsidx16=pairsidx16[c],
        )
        in_maps.append(m)
    return meta, in_maps


def _build(meta, ncores=NCORES, single=False, dump=False):
    from concourse import bass, mybir, tile, bacc

    N, H, R = meta["N"], meta["H"], meta["R"]
    NOWN, TILES, TC, Q = meta["NOWN"], meta["TILES"], meta["TC"], meta["Q"]
    T2 = meta["T2"]
    KC, MC, GW = meta["KC"], meta["MC"], meta["GW"]
    nch, chunk_start = meta["nch"], meta["chunk_start"]
    groups, gmap = meta["groups"], meta["gmap"]
    rng_lo, rng_w = meta["rng_lo"], meta["rng_w"]
    f32 = mybir.dt.float32
    f16 = mybir.dt.float16
    i16 = mybir.dt.int16

    nc = bacc.Bacc(
        "TRN2", target_bir_lowering=False, debug=False,
        num_devices=(1 if single else ncores),
        dynamic_dma_scratch_size=65536,
    )

    h0_t = nc.dram_tensor("h0", [N, H], f16, kind="ExternalInput")
    idx16_t = nc.dram_tensor("idx16", [P, GW], i16, kind="ExternalInput")
    dstlocT_t = nc.dram_tensor("dstlocT", [P, TC], f32, kind="ExternalInput")
    normT_t = nc.dram_tensor("normT", [P, TC], f32, kind="ExternalInput")
    ownidx_t = nc.dram_tensor("ownidx16", [P, T2 * 16], i16, kind="ExternalInput")
    pairsidx_t = nc.dram_tensor("pairsidx16", [P, Q * 16], i16, kind="ExternalInput")
    iota_t = nc.dram_tensor("iota", [P, TILE2], f16, kind="ExternalInput")
    wblk_t = nc.dram_tensor("wblk", [P, 2 * R * 2 * P], f16, kind="ExternalInput")
    loopw_t = nc.dram_tensor("loopw", [P, 2 * 2 * H], f16, kind="ExternalInput")
    biasbc_t = nc.dram_tensor("biasbc", [P, 2 * H], f32, kind="ExternalInput")
    fc1_t = nc.dram_tensor("fc1", [P, KC * MC * P], f16, kind="ExternalInput")
    fc1b_t = nc.dram_tensor("fc1b", [P, MC], f32, kind="ExternalInput")
    fc2_t = nc.dram_tensor("fc2", [P, MC], f16, kind="ExternalInput")
    out_t = nc.dram_tensor("out", [Q * P, 1], f32, kind="ExternalOutput")

    with tile.TileContext(nc) as tc:
        with (
            tc.tile_pool(name="const", bufs=1) as cp,
            tc.tile_pool(name="work", bufs=10) as wp,
            tc.tile_pool(name="ps", bufs=1, space="PSUM") as pp,
        ):
            # ---- DRAM internals; one whole-layer AllGather per layer ----
            h1_full = nc.dram_tensor("h1_full", [N, H], f16, kind="Internal").ap()
            h2_full = nc.dram_tensor("h2_full", [N, H], f16, kind="Internal").ap()
            agin = {}
            agout = {}
            for li in (1, 2):
                agin[li] = nc.dram_tensor(
                    f"h{li}_agin", [NOWN, H], f16, kind="Internal"
                ).ap()
                agout[li] = nc.dram_tensor(
                    f"h{li}_agout", [N, H], f16, kind="Internal",
                    addr_space="Shared",
                ).ap()

            # ---- resident constants ----
            idx16 = cp.tile([P, GW], i16, name="idx16")
            nc.sync.dma_start(idx16[:], idx16_t.ap()[:])
            dstlocT = cp.tile([P, TC], f32, name="dstlocT")
            nc.sync.dma_start(dstlocT[:], dstlocT_t.ap()[:])
            normT = cp.tile([P, TC], f32, name="normT")
            nc.sync.dma_start(normT[:], normT_t.ap()[:])
            ownidx = cp.tile([P, T2 * 16], i16, name="ownidx")
            nc.sync.dma_start(ownidx[:], ownidx_t.ap()[:])
            pairsidx = cp.tile([P, Q * 16], i16, name="pairsidx")
            nc.sync.dma_start(pairsidx[:], pairsidx_t.ap()[:])
            iota_sb = cp.tile([P, TILE2], f16, name="iota_sb")
            nc.sync.dma_start(iota_sb[:], iota_t.ap()[:])
            wblk_sb = cp.tile([P, 2 * R * 2 * P], f16, name="wblk_sb")
            nc.sync.dma_start(wblk_sb[:], wblk_t.ap()[:])
            loopw_sb = cp.tile([P, 2 * 2 * H], f16, name="loopw_sb")
            nc.sync.dma_start(loopw_sb[:], loopw_t.ap()[:])
            biasbc_sb = cp.tile([P, 2 * H], f32, name="biasbc_sb")
            nc.sync.dma_start(biasbc_sb[:], biasbc_t.ap()[:])
            fc1_sb = cp.tile([P, KC * MC * P], f16, name="fc1_sb")
            nc.sync.dma_start(fc1_sb[:], fc1_t.ap()[:])
            fc1b_sb = cp.tile([P, MC], f32, name="fc1b_sb")
            nc.sync.dma_start(fc1b_sb[:], fc1b_t.ap()[:])
            fc2_sb = cp.tile([P, MC], f16, name="fc2_sb")
            nc.sync.dma_start(fc2_sb[:], fc2_t.ap()[:])

            def wblk_ap(l, r, h):
                o = ((l * R + r) * 2 + h) * P
                return wblk_sb[:, o : o + P]

            def loopw_ap(l, h):
                o = (l * 2 + h) * H
                return loopw_sb[:, o : o + H]

            def emit_ag(li, h_full):
                if single:
                    nc.sync.dma_start(h_full[0:NOWN, :], agin[li][:])
                    return
                nc.gpsimd.collective_compute(
                    "AllGather", mybir.AluOpType.bypass,
                    replica_groups=[list(range(ncores))],
                    ins=[agin[li]], outs=[agout[li]],
                )
                nc.sync.dma_start(h_full[:], agout[li][:])

            copy_flip = [0]

            def layer(l, xsrc_ap, li, h_full_out):
                for t2 in range(T2):
                    # self-loop x^T via transpose gather: [if%128, if//128, dst]
                    xT = wp.tile([P, 2 * TILE2], f16, name="xT", tag="xT", bufs=3)
                    nc.gpsimd.dma_gather(
                        out_ap=xT[:].rearrange("p (j d) -> p j d", d=TILE2),
                        in_ap=xsrc_ap,
                        idxs_ap=ownidx[:, t2 * 16 : (t2 + 1) * 16],
                        num_idxs=TILE2, num_idxs_reg=TILE2, elem_size=H,
                        transpose=True,
                    )
                    subs = [st for st in (2 * t2, 2 * t2 + 1) if st < TILES]
                    # issue this t2's edge gathers
                    t2_groups = [
                        g for g in groups
                        if chunk_start[t2 * R] <= g[0]
                        and g[0] < chunk_start[t2 * R] + nch[t2 * R : (t2 + 1) * R].sum()
                    ]
                    gtiles = {}
                    for c0, k, off in t2_groups:
                        xg = wp.tile([P, GK * H], f16, name="xg", tag="xg", bufs=6)
                        nc.gpsimd.dma_gather(
                            out_ap=xg[:, : k * H].rearrange("p (k h) -> p k h", h=H),
                            in_ap=xsrc_ap,
                            idxs_ap=idx16[:, off : off + k * 8],
                            num_idxs=k * P, num_idxs_reg=k * P, elem_size=H,
                        )
                        gtiles[c0] = (xg, c0)

                    rels = [r for r in range(R) if nch[t2 * R + r] > 0]
                    msg = {}
                    for si_, st in enumerate(subs):
                        msg[si_] = pp.tile(
                            [P, TILE2], f32, name=f"msg{si_}", tag=f"msg{si_}", bufs=2
                        )
                        for j in range(2):
                            nc.tensor.matmul(
                                msg[si_][:],
                                lhsT=xT[:, j * TILE2 + si_ * P : j * TILE2 + si_ * P + P],
                                rhs=loopw_ap(l, j),
                                start=(j == 0), stop=(j == 1 and not rels),
                            )
                    pending = []  # [(aggsb, r), ...] awaiting relation GEMM
                    for ri, r in enumerate(rels):
                        cell = t2 * R + r
                        cs = int(chunk_start[cell])
                        n = int(nch[cell])
                        aggT = pp.tile(
                            [P, 2 * TILE2], f32, name="aggT", tag="aggT", bufs=4
                        )
                        for ci in range(n):
                            col = cs + ci
                            gid, joff = gmap[col]
                            xg, gc0 = gtiles[groups[gid][0]]
                            a = int(rng_lo[col])
                            w = int(rng_w[col])
                            S = wp.tile([P, TILE2], f16, name="S", tag="S", bufs=12)
                            nc.vector.tensor_scalar(
                                out=S[:, :w], in0=iota_sb[:, a : a + w],
                                scalar1=dstlocT[:, col : col + 1],
                                scalar2=normT[:, col : col + 1],
                                op0=mybir.AluOpType.is_equal, op1=mybir.AluOpType.mult,
                            )
                            for h in range(2):
                                nc.tensor.matmul(
                                    aggT[:, h * TILE2 + a : h * TILE2 + a + w],
                                    lhsT=xg[:, joff * H + h * P : joff * H + h * P + P],
                                    rhs=S[:, :w], start=True, stop=True,
                                )
                        aggsb = wp.tile([P, 2 * TILE2], f16, name="aggsb", tag="aggsb", bufs=6)
                        if copy_flip[0] % 4 == 0:
                            nc.vector.tensor_copy(aggsb[:], aggT[:])
                        else:
                            nc.scalar.copy(aggsb[:], aggT[:])
                        copy_flip[0] += 1

                        pending.append((aggsb, r))
                        if len(pending) > 2:
                            psb, pr = pending.pop(0)
                            for si_, st in enumerate(subs):
                                for h in range(2):
                                    nc.tensor.matmul(
                                        msg[si_][:, h * P : (h + 1) * P],
                                        lhsT=psb[:, h * TILE2 + si_ * P : h * TILE2 + si_ * P + P],
                                        rhs=wblk_ap(l, pr, h), start=False, stop=False,
                                    )

                    # drain remaining relation GEMMs; last one closes the chain
                    for pi_, (psb, pr) in enumerate(pending):
                        last = pi_ == len(pending) - 1
                        for si_, st in enumerate(subs):
                            for h in range(2):
                                nc.tensor.matmul(
                                    msg[si_][:, h * P : (h + 1) * P],
                                    lhsT=psb[:, h * TILE2 + si_ * P : h * TILE2 + si_ * P + P],
                                    rhs=wblk_ap(l, pr, h), start=False, stop=last,
                                )

                    for si_, st in enumerate(subs):
                        out_sb = wp.tile([P, H], f16, name="outsb", tag="outsb", bufs=4)
                        nc.vector.tensor_tensor(
                            out=out_sb[:], in0=msg[si_][:],
                            in1=biasbc_sb[:, l * H : (l + 1) * H],
                            op=mybir.AluOpType.add,
                        )
                        rows = min(P, NOWN - st * P)
                        nc.sync.dma_start(
                            agin[li][st * P : st * P + rows, :], out_sb[:rows, :]
                        )
                        if st == TILES - 1:
                            emit_ag(li, h_full_out)

            layer(0, h0_t.ap()[:], 1, h1_full)
            layer(1, h1_full[:], 2, h2_full)
            if dump:
                h1d = nc.dram_tensor("h1dump", [N, H], f16, kind="ExternalOutput")
                nc.sync.dma_start(h1d.ap()[:], h1_full[:])
                h2d = nc.dram_tensor("h2dump", [N, H], f16, kind="ExternalOutput")
                nc.sync.dma_start(h2d.ap()[:], h2_full[:])

            # ---- MLP head over this core's Q*P pairs (transposed layout) ----
            for q in range(Q):
                xpT = wp.tile([P, 2 * 2 * P], f16, name="xpT", tag="xpT", bufs=2)
                nc.gpsimd.dma_gather(
                    out_ap=xpT[:].rearrange("p (j d) -> p j d", d=2 * P),
                    in_ap=h2_full[:],
                    idxs_ap=pairsidx[:, q * 16 : (q + 1) * 16],
                    num_idxs=2 * P, num_idxs_reg=2 * P, elem_size=H,
                    transpose=True,
                )

                # xcat chunk ifc -> slice of xpT: drug (j, 0:128), target (j, 128:256)
                def xc(ifc):
                    j = ifc % 2
                    half = 0 if ifc < 2 else P
                    return xpT[:, j * 2 * P + half : j * 2 * P + half + P]

                yTr = wp.tile([P, MC * P], f16, name="yTr", tag="yTr", bufs=2)
                z = pp.tile([P, TILE2], f32, name="z", tag="msg0", bufs=2)
                for m in range(MC):
                    yT = pp.tile([P, 2 * TILE2], f32, name="yT", tag="aggT", bufs=4)
                    for k in range(KC):
                        nc.tensor.matmul(
                            yT[:, 0:P],
                            lhsT=fc1_sb[:, (k * MC + m) * P : (k * MC + m + 1) * P],
                            rhs=xc(k), start=(k == 0), stop=(k == KC - 1),
                        )
                    nc.scalar.activation(
                        yTr[:, m * P : (m + 1) * P], yT[:, 0:P],
                        mybir.ActivationFunctionType.Relu,
                        bias=fc1b_sb[:, m : m + 1], scale=1.0,
                    )
                    nc.tensor.matmul(
                        z[0:1, 0:P], lhsT=fc2_sb[:, m : m + 1],
                        rhs=yTr[:, m * P : (m + 1) * P],
                        start=(m == 0), stop=(m == MC - 1),
                    )
                zs = wp.tile([1, P], f32, name="zs", tag="zs", bufs=2)
                nc.scalar.activation(
                    zs[:], z[0:1, 0:P], mybir.ActivationFunctionType.Sigmoid,
                    bias=meta["fc2b"], scale=1.0,
                )
                nc.sync.dma_start(out_t.ap()[q * P : (q + 1) * P, :], zs[:])
    return nc


_NC_CACHE = []


def kernel(**inputs):
    from concourse import bass_utils

    meta, in_maps = _preprocess(inputs)
    key = (meta["N"], meta["H"], meta["R"], meta["TC"], meta["Q"],
           tuple(int(x) for x in meta["nch"]))
    if _NC_CACHE and _NC_CACHE[0][0] == key:
        nc = _NC_CACHE[0][1]
    else:
        nc = _build(meta)
        nc.compile()
        _NC_CACHE[:] = [(key, nc)]
    res = bass_utils.run_bass_kernel_spmd(nc, in_maps, core_ids=list(range(NCORES)))
    out = np.concatenate([res.results[c]["out"] for c in range(NCORES)], axis=0)
    return out.astype(np.float32)


# revision 27
# speedup vs baseline: 4.2256x; 1.0179x over previous
"""Trainium2 Bass kernel for the DTI R-GCN (bdd) model, 8 NeuronCores.

Strategy (SPMD, one program, per-core data):
  - dst-shard the graph: core c owns nodes [c*2500, (c+1)*2500); host routes
    each edge to its dst owner and buckets it into (dst-tile, relation) cells,
    dst tiles of 256 nodes, edge chunks of 128 (the PE contraction dim).
  - all tensors on the PE path are fp16 (1 cycle/row vs 4 for fp32; PSUM
    accumulation stays fp32), features are gathered as 512B fp16 rows.
  - gathers use the batched int16-index dma_gather (up to 8 chunks = 1024
    rows per instruction) to amortize the SWDGE fixed cost; pad slots point
    at row 0 with norm 0 so they contribute nothing.
  - per 128-edge chunk: scatter matrix S[e, d] = norm_e * (iota == dstloc_e)
    built in one fp16 tensor_scalar (DVE 2x mode), then 2 matmuls accumulate
    the transposed per-(tile, rel) aggregate aggT[if_half, dst] in PSUM.
  - per cell: one combined [P,512] PSUM->SBUF fp16 copy (alternating
    DVE/Act), then 4 relation-block matmuls accumulate msg[dst, of] in PSUM;
    the relation GEMM lags one relation behind the aggregation so the PE
    never waits on the copy.
  - self-loop x^T comes straight from a transpose-mode gather of the node
    table (no PE transposes), accumulated into the same msg PSUM.
  - layer output (own 2500 rows) is AllGather'd piecewise so every core has
    the full [20000, 256] fp16 activations for the next layer's gathers.
  - MLP head is data-parallel over pairs (512 per core): one transpose-mode
    gather delivers xcat^T per 128-pair block, so fc1/fc2 run with weights
    as natural lhsT and no transposes.
"""
import sys

sys.path.insert(0, "/opt/trn_rl_repo")
import numpy as np

P = 128
NCORES = 8
TILE2 = 2 * P  # dst nodes per aggregation cell
GK = 8  # max chunks per dma_gather


def _wrap16(arr):
    """int sequence [n] (n % 16 == 0) -> wrapped idx layout [128, n//16]:
    idx i at (i % 16, i // 16), replicated across the 8 partition groups."""
    n = arr.shape[0]
    blk = arr.reshape(n // 16, 16).T
    return np.tile(blk, (8, 1)).astype(np.int16)


def _preprocess(inputs, ncores=NCORES):
    node_ids = np.asarray(inputs["node_ids"])
    src = np.asarray(inputs["src"])
    dst = np.asarray(inputs["dst"])
    etype = np.asarray(inputs["etype"])
    norm = np.asarray(inputs["norm"]).reshape(-1)
    emb = np.asarray(inputs["emb"], dtype=np.float32)
    drugs = np.asarray(inputs["drugs_index"])
    targets = np.asarray(inputs["targets_index"])

    N = node_ids.shape[0]
    H = emb.shape[1]
    R = int(inputs["w1"].shape[0])
    PAIRS = drugs.shape[0]
    assert N % ncores == 0 and PAIRS % ncores == 0
    NOWN = N // ncores
    TILES = -(-NOWN // P)
    PPC = PAIRS // ncores
    assert PPC % P == 0
    Q = PPC // P

    T2 = -(-NOWN // TILE2)
    NCELL = T2 * R
    owner = dst // NOWN
    d_local = dst - owner * NOWN
    t_of_e = d_local // TILE2
    dstloc_of_e = (d_local % TILE2).astype(np.float32)
    cell_of_e = t_of_e * R + etype

    # per-(cell, dstloc) max-over-cores counts -> disjoint dst-range chunks:
    # each chunk owns a dst range [a, b) of its cell with max-core count <= P,
    # so each dst column is scatter-written exactly once per cell.
    fine = np.zeros((ncores, NCELL * TILE2), np.int64)
    fine_idx = cell_of_e * TILE2 + (d_local % TILE2)
    for c in range(ncores):
        fine[c] = np.bincount(fine_idx[owner == c], minlength=NCELL * TILE2)
    finec = fine.reshape(ncores, NCELL, TILE2)
    counts = finec.sum(axis=2)
    nch = np.zeros(NCELL, np.int64)
    rng_lo = []  # per chunk col: range start (within cell tile)
    rng_w = []   # per chunk col: range width
    for cell in range(NCELL):
        if counts[:, cell].max() == 0:
            continue
        a = 0
        run = np.zeros(ncores, np.int64)
        n = 0
        for d in range(TILE2):
            cnt = finec[:, cell, d]
            assert cnt.max() <= P, "single dst exceeds chunk capacity"
            if (run + cnt).max() > P:
                rng_lo.append(a)
                rng_w.append(d - a)
                a = d
                run[:] = 0
                n += 1
            run += cnt
        rng_lo.append(a)
        rng_w.append(TILE2 - a)
        nch[cell] = n + 1
    chunk_start = np.zeros(NCELL, np.int64)
    chunk_start[1:] = np.cumsum(nch)[:-1]
    TC = int(nch.sum())
    rng_lo = np.asarray(rng_lo, np.int64)
    rng_w = np.asarray(rng_w, np.int64)

    srcT = np.zeros((ncores, P, TC), np.int16)  # pad slots -> row 0
    dstlocT = np.zeros((ncores, P, TC), np.float32)
    normT = np.zeros((ncores, P, TC), np.float32)
    dl256 = d_local % TILE2
    for c in range(ncores):
        m = owner == c
        eidx = np.where(m)[0]
        # order edges by (cell, dstloc) then assign to the chunk owning the range
        order = np.lexsort((dl256[eidx], cell_of_e[eidx]))
        eidx = eidx[order]
        cell = cell_of_e[eidx]
        dl = dl256[eidx]
        # chunk col for each edge: chunk_start[cell] + searchsorted within ranges
        col = np.empty(len(eidx), np.int64)
        for cc in np.unique(cell):
            mm = cell == cc
            cs = int(chunk_start[cc])
            ne = int(nch[cc])
            lo = rng_lo[cs : cs + ne]
            col[mm] = cs + np.searchsorted(lo, dl[mm], side="right") - 1
        # slot within chunk = running rank per chunk col
        uniq, inv, cnts = np.unique(col, return_inverse=True, return_counts=True)
        starts = np.zeros(len(uniq), np.int64)
        starts[1:] = np.cumsum(cnts)[:-1]
        part = np.arange(len(eidx)) - starts[inv]
        assert part.max() < P
        srcT[c, part, col] = src[eidx]
        dstlocT[c, part, col] = dstloc_of_e[eidx]
        normT[c, part, col] = norm[eidx]

    # gather groups: per t2, consecutive chunk columns in groups of <= GK
    groups = []  # (col0, k, goff)
    gmap = {}  # chunk col -> (gid, offset within group)
    goff = 0
    for t2 in range(T2):
        cs = int(chunk_start[t2 * R])
        ce = cs + int(nch[t2 * R : (t2 + 1) * R].sum())
        for c0 in range(cs, ce, GK):
            k = min(GK, ce - c0)
            gid = len(groups)
            groups.append((c0, k, goff))
            for j in range(k):
                gmap[c0 + j] = (gid, j)
            goff += k * 8
    GW = goff

    idx16 = np.zeros((ncores, P, GW), np.int16)
    for c in range(ncores):
        for c0, k, off in groups:
            vals = srcT[c, :, c0 : c0 + k]  # [P, k]
            arr = vals.T.reshape(k * P)
            idx16[c, :, off : off + k * 8] = _wrap16(arr)

    # self-loop transpose-gather indices: per t2, own rows (pad -> own row 0)
    ownidx16 = np.zeros((ncores, P, T2 * 16), np.int16)
    for c in range(ncores):
        for t2 in range(T2):
            j = t2 * TILE2 + np.arange(TILE2)
            j = np.where(j < NOWN, j, 0) + c * NOWN
            ownidx16[c, :, t2 * 16 : (t2 + 1) * 16] = _wrap16(j)

    # head transpose-gather indices: per q, [drugs block | targets block]
    pairsidx16 = np.zeros((ncores, P, Q * 16), np.int16)
    for c in range(ncores):
        d_c = drugs[c * PPC : (c + 1) * PPC]
        t_c = targets[c * PPC : (c + 1) * PPC]
        for q in range(Q):
            arr = np.concatenate([d_c[q * P : (q + 1) * P], t_c[q * P : (q + 1) * P]])
            pairsidx16[c, :, q * 16 : (q + 1) * 16] = _wrap16(arr)

    # host-side embedding lookup, fp16 node table
    h0 = emb[node_ids].astype(np.float16)  # [N, H]

    # relation block weights as rhs [if_local, of_local] per (layer, rel, half)
    B = int(inputs["w1"].shape[1])
    si = H // B
    hb = P // si  # blocks per half
    wblk = np.zeros((2, R, 2, P, P), np.float16)
    for l, W in enumerate([inputs["w1"], inputs["w2"]]):
        W = np.asarray(W, np.float32)
        for r in range(R):
            for h in range(2):
                for bb in range(hb):
                    b = hb * h + bb
                    wblk[l, r, h, bb * si : (bb + 1) * si, bb * si : (bb + 1) * si] = W[r, b]
    wblk_in = wblk.transpose(3, 0, 1, 2, 4).reshape(P, 2 * R * 2 * P).copy()

    # self-loop weights as lhsT-compatible rhs [if_half rows, of=H] per (l, h)
    loopw = np.stack(
        [np.asarray(inputs["loop_w1"], np.float32), np.asarray(inputs["loop_w2"], np.float32)]
    ).astype(np.float16)  # [2, H, H]
    loopw_in = loopw.reshape(2, 2, P, H).transpose(2, 0, 1, 3).reshape(P, 2 * 2 * H).copy()

    bias_in = np.concatenate(
        [
            np.tile(np.asarray(inputs["b1"], np.float32)[None, :], (P, 1)),
            np.tile(np.asarray(inputs["b2"], np.float32)[None, :], (P, 1)),
        ],
        axis=1,
    )  # [P, 2H] f32

    d2 = 2 * H
    KC = d2 // P
    MC = d2 // P
    # fc1 blocks as natural lhsT [if128, of128] per (ifc, ofc)
    fc1_in = (
        np.asarray(inputs["fc1_W"], np.float16)
        .reshape(KC, P, MC, P)
        .transpose(1, 0, 2, 3)
        .reshape(P, KC * MC * P)
        .copy()
    )
    fc1b_in = np.asarray(inputs["fc1_b"], np.float32).reshape(MC, P).T.copy()
    fc2_in = np.asarray(inputs["fc2_W"], np.float16).reshape(MC, P).T.copy()
    fc2b = float(np.asarray(inputs["fc2_b"]).reshape(-1)[0])

    iota = np.tile(np.arange(TILE2, dtype=np.float16), (P, 1))

    zero_bias = (
        float(np.abs(np.asarray(inputs["b1"])).max()) == 0.0
        and float(np.abs(np.asarray(inputs["b2"])).max()) == 0.0
    )
    meta = dict(
        N=N, H=H, R=R, NOWN=NOWN, TILES=TILES, T2=T2, NCELL=NCELL, TC=TC, Q=Q,
        KC=KC, MC=MC, GW=GW, nch=nch, chunk_start=chunk_start, groups=groups,
        gmap=gmap, rng_lo=rng_lo, rng_w=rng_w, fc2b=fc2b, zero_bias=zero_bias,
    )
    shared = dict(
        h0=h0, iota=iota, wblk=wblk_in, loopw=loopw_in, biasbc=bias_in,
        fc1=fc1_in, fc1b=fc1b_in, fc2=fc2_in,
    )
    in_maps = []
    for c in range(ncores):
        m = dict(shared)
        m.update(
            idx16=idx16[c], dstlocT=dstlocT[c], normT=normT[c],
            ownidx16=ownidx16[c], pairsidx16=pairsidx16[c],
        )
        in_maps.append(m)
    return meta, in_maps


def _build(meta, ncores=NCORES, single=False, dump=False):
    from concourse import bass, mybir, tile, bacc

    N, H, R = meta["N"], meta["H"], meta["R"]
    NOWN, TILES, TC, Q = meta["NOWN"], meta["TILES"], meta["TC"], meta["Q"]
    T2 = meta["T2"]
    KC, MC, GW = meta["KC"], meta["MC"], meta["GW"]
    nch, chunk_start = meta["nch"], meta["chunk_start"]
    groups, gmap = meta["groups"], meta["gmap"]
    rng_lo, rng_w = meta["rng_lo"], meta["rng_w"]
    f32 = mybir.dt.float32
    f16 = mybir.dt.float16
    i16 = mybir.dt.int16

    nc = bacc.Bacc(
        "TRN2", target_bir_lowering=False, debug=False,
        num_devices=(1 if single else ncores),
        dynamic_dma_scratch_size=65536,
    )

    h0_t = nc.dram_tensor("h0", [N, H], f16, kind="ExternalInput")
    idx16_t = nc.dram_tensor("idx16", [P, GW], i16, kind="ExternalInput")
    dstlocT_t = nc.dram_tensor("dstlocT", [P, TC], f32, kind="ExternalInput")
    normT_t = nc.dram_tensor("normT", [P, TC], f32, kind="ExternalInput")
    ownidx_t = nc.dram_tensor("ownidx16", [P, T2 * 16], i16, kind="ExternalInput")
    pairsidx_t = nc.dram_tensor("pairsidx16", [P, Q * 16], i16, kind="ExternalInput")
    iota_t = nc.dram_tensor("iota", [P, TILE2], f16, kind="ExternalInput")
    wblk_t = nc.dram_tensor("wblk", [P, 2 * R * 2 * P], f16, kind="ExternalInput")
    loopw_t = nc.dram_tensor("loopw", [P, 2 * 2 * H], f16, kind="ExternalInput")
    biasbc_t = nc.dram_tensor("biasbc", [P, 2 * H], f32, kind="ExternalInput")
    fc1_t = nc.dram_tensor("fc1", [P, KC * MC * P], f16, kind="ExternalInput")
    fc1b_t = nc.dram_tensor("fc1b", [P, MC], f32, kind="ExternalInput")
    fc2_t = nc.dram_tensor("fc2", [P, MC], f16, kind="ExternalInput")
    out_t = nc.dram_tensor("out", [Q * P, 1], f32, kind="ExternalOutput")

    with tile.TileContext(nc) as tc:
        with (
            tc.tile_pool(name="const", bufs=1) as cp,
            tc.tile_pool(name="work", bufs=10) as wp,
            tc.tile_pool(name="ps", bufs=1, space="PSUM") as pp,
        ):
            # ---- DRAM internals; AllGather split into piece collectives ----
            tpp = max(1, -(-TILES // 4))  # 128-row tiles per AG piece
            pieces = []  # (row0, nrows)
            for p0 in range(0, TILES, tpp):
                row0 = p0 * P
                nrows = min(NOWN, (p0 + tpp) * P) - row0
                if nrows > 0:
                    pieces.append((row0, nrows))
            h1_full = nc.dram_tensor("h1_full", [N, H], f16, kind="Internal").ap()
            h2_full = nc.dram_tensor("h2_full", [N, H], f16, kind="Internal").ap()
            agin = {}
            agout = {}
            for li in (1, 2):
                agin[li] = nc.dram_tensor(
                    f"h{li}_agin", [NOWN, H], f16, kind="Internal"
                ).ap()
                for pi, (row0, nrows) in enumerate(pieces):
                    agout[(li, pi)] = nc.dram_tensor(
                        f"h{li}_agout{pi}", [ncores * nrows, H], f16,
                        kind="Internal", addr_space="Shared",
                    ).ap()

            # ---- resident constants ----
            idx16 = cp.tile([P, GW], i16, name="idx16")
            nc.sync.dma_start(idx16[:], idx16_t.ap()[:])
            dstlocT = cp.tile([P, TC], f32, name="dstlocT")
            nc.sync.dma_start(dstlocT[:], dstlocT_t.ap()[:])
            normT = cp.tile([P, TC], f32, name="normT")
            nc.sync.dma_start(normT[:], normT_t.ap()[:])
            ownidx = cp.tile([P, T2 * 16], i16, name="ownidx")
            nc.sync.dma_start(ownidx[:], ownidx_t.ap()[:])
            pairsidx = cp.tile([P, Q * 16], i16, name="pairsidx")
            nc.sync.dma_start(pairsidx[:], pairsidx_t.ap()[:])
            iota_sb = cp.tile([P, TILE2], f16, name="iota_sb")
            nc.sync.dma_start(iota_sb[:], iota_t.ap()[:])
            wblk_sb = cp.tile([P, 2 * R * 2 * P], f16, name="wblk_sb")
            nc.sync.dma_start(wblk_sb[:], wblk_t.ap()[:])
            loopw_sb = cp.tile([P, 2 * 2 * H], f16, name="loopw_sb")
            nc.sync.dma_start(loopw_sb[:], loopw_t.ap()[:])
            biasbc_sb = cp.tile([P, 2 * H], f32, name="biasbc_sb")
            nc.sync.dma_start(biasbc_sb[:], biasbc_t.ap()[:])
            fc1_sb = cp.tile([P, KC * MC * P], f16, name="fc1_sb")
            nc.sync.dma_start(fc1_sb[:], fc1_t.ap()[:])
            fc1b_sb = cp.tile([P, MC], f32, name="fc1b_sb")
            nc.sync.dma_start(fc1b_sb[:], fc1b_t.ap()[:])
            fc2_sb = cp.tile([P, MC], f16, name="fc2_sb")
            nc.sync.dma_start(fc2_sb[:], fc2_t.ap()[:])

            def wblk_ap(l, r, h):
                o = ((l * R + r) * 2 + h) * P
                return wblk_sb[:, o : o + P]

            def loopw_ap(l, h):
                o = (l * 2 + h) * H
                return loopw_sb[:, o : o + H]

            def emit_ag_piece(li, pi, h_full):
                row0, nrows = pieces[pi]
                if single:
                    nc.sync.dma_start(
                        h_full[row0 : row0 + nrows, :],
                        agin[li][row0 : row0 + nrows, :],
                    )
                    return
                nc.gpsimd.collective_compute(
                    "AllGather", mybir.AluOpType.bypass,
                    replica_groups=[list(range(ncores))],
                    ins=[agin[li][row0 : row0 + nrows, :]],
                    outs=[agout[(li, pi)]],
                )
                src_ap = agout[(li, pi)].rearrange("(c n) h -> c n h", c=ncores)
                dst_ap = h_full.rearrange("(c n) h -> c n h", c=ncores)[
                    :, row0 : row0 + nrows, :
                ]
                nc.sync.dma_start(dst_ap, src_ap)

            copy_flip = [0]

            def layer(l, xsrc_ap, li, h_full_out):
                for t2 in range(T2):
                    # self-loop x^T via transpose gather: [if%128, if//128, dst]
                    xT = wp.tile([P, 2 * TILE2], f16, name="xT", tag="xT", bufs=3)
                    nc.gpsimd.dma_gather(
                        out_ap=xT[:].rearrange("p (j d) -> p j d", d=TILE2),
                        in_ap=xsrc_ap,
                        idxs_ap=ownidx[:, t2 * 16 : (t2 + 1) * 16],
                        num_idxs=TILE2, num_idxs_reg=TILE2, elem_size=H,
                        transpose=True,
                    )
                    subs = [st for st in (2 * t2, 2 * t2 + 1) if st < TILES]
                    # issue this t2's edge gathers
                    t2_groups = [
                        g for g in groups
                        if chunk_start[t2 * R] <= g[0]
                        and g[0] < chunk_start[t2 * R] + nch[t2 * R : (t2 + 1) * R].sum()
                    ]
                    gtiles = {}
                    for c0, k, off in t2_groups:
                        xg = wp.tile([P, GK * H], f16, name="xg", tag="xg", bufs=6)
                        nc.gpsimd.dma_gather(
                            out_ap=xg[:, : k * H].rearrange("p (k h) -> p k h", h=H),
                            in_ap=xsrc_ap,
                            idxs_ap=idx16[:, off : off + k * 8],
                            num_idxs=k * P, num_idxs_reg=k * P, elem_size=H,
                        )
                        gtiles[c0] = (xg, c0)

                    rels = [r for r in range(R) if nch[t2 * R + r] > 0]
                    msg = {}
                    for si_, st in enumerate(subs):
                        msg[si_] = pp.tile(
                            [P, TILE2], f32, name=f"msg{si_}", tag=f"msg{si_}", bufs=2
                        )
                        for j in range(2):
                            nc.tensor.matmul(
                                msg[si_][:],
                                lhsT=xT[:, j * TILE2 + si_ * P : j * TILE2 + si_ * P + P],
                                rhs=loopw_ap(l, j),
                                start=(j == 0), stop=(j == 1 and not rels),
                            )
                    pending = []  # [(aggsb, r), ...] awaiting relation GEMM
                    for ri, r in enumerate(rels):
                        cell = t2 * R + r
                        cs = int(chunk_start[cell])
                        n = int(nch[cell])
                        aggT = pp.tile(
                            [P, 2 * TILE2], f32, name="aggT", tag="aggT", bufs=4
                        )
                        for ci in range(n):
                            col = cs + ci
                            gid, joff = gmap[col]
                            xg, gc0 = gtiles[groups[gid][0]]
                            a = int(rng_lo[col])
                            w = int(rng_w[col])
                            S = wp.tile([P, TILE2], f16, name="S", tag="S", bufs=12)
                            nc.vector.tensor_scalar(
                                out=S[:, :w], in0=iota_sb[:, a : a + w],
                                scalar1=dstlocT[:, col : col + 1],
                                scalar2=normT[:, col : col + 1],
                                op0=mybir.AluOpType.is_equal, op1=mybir.AluOpType.mult,
                            )
                            for h in range(2):
                                nc.tensor.matmul(
                                    aggT[:, h * TILE2 + a : h * TILE2 + a + w],
                                    lhsT=xg[:, joff * H + h * P : joff * H + h * P + P],
                                    rhs=S[:, :w], start=True, stop=True,
                                )
                        aggsb = wp.tile([P, 2 * TILE2], f16, name="aggsb", tag="aggsb", bufs=6)
                        if copy_flip[0] % 8 in (0, 4):
                            nc.vector.tensor_copy(aggsb[:], aggT[:])
                        else:
                            nc.scalar.copy(aggsb[:], aggT[:])
                        copy_flip[0] += 1

                        pending.append((aggsb, r))
                        if len(pending) > 2:
                            psb, pr = pending.pop(0)
                            for si_, st in enumerate(subs):
                                for h in range(2):
                                    nc.tensor.matmul(
                                        msg[si_][:, h * P : (h + 1) * P],
                                        lhsT=psb[:, h * TILE2 + si_ * P : h * TILE2 + si_ * P + P],
                                        rhs=wblk_ap(l, pr, h), start=False, stop=False,
                                    )

                    # drain remaining relation GEMMs; last one closes the chain
                    for pi_, (psb, pr) in enumerate(pending):
                        last = pi_ == len(pending) - 1
                        for si_, st in enumerate(subs):
                            for h in range(2):
                                nc.tensor.matmul(
                                    msg[si_][:, h * P : (h + 1) * P],
                                    lhsT=psb[:, h * TILE2 + si_ * P : h * TILE2 + si_ * P + P],
                                    rhs=wblk_ap(l, pr, h), start=False, stop=last,
                                )

                    for si_, st in enumerate(subs):
                        out_sb = wp.tile([P, H], f16, name="outsb", tag="outsb", bufs=4)
                        if meta["zero_bias"]:
                            nc.scalar.copy(out_sb[:], msg[si_][:])
                        else:
                            nc.vector.tensor_tensor(
                                out=out_sb[:], in0=msg[si_][:],
                                in1=biasbc_sb[:, l * H : (l + 1) * H],
                                op=mybir.AluOpType.add,
                            )
                        rows = min(P, NOWN - st * P)
                        nc.sync.dma_start(
                            agin[li][st * P : st * P + rows, :], out_sb[:rows, :]
                        )
                        if st == TILES - 1 or (st + 1) % tpp == 0:
                            emit_ag_piece(li, st // tpp, h_full_out)

            layer(0, h0_t.ap()[:], 1, h1_full)
            layer(1, h1_full[:], 2, h2_full)
            if dump:
                h1d = nc.dram_tensor("h1dump", [N, H], f16, kind="ExternalOutput")
                nc.sync.dma_start(h1d.ap()[:], h1_full[:])
                h2d = nc.dram_tensor("h2dump", [N, H], f16, kind="ExternalOutput")
                nc.sync.dma_start(h2d.ap()[:], h2_full[:])

            # ---- MLP head over this core's Q*P pairs (transposed layout) ----
            for q in range(Q):
                xpT = wp.tile([P, 2 * 2 * P], f16, name="xpT", tag="xpT", bufs=2)
                nc.gpsimd.dma_gather(
                    out_ap=xpT[:].rearrange("p (j d) -> p j d", d=2 * P),
                    in_ap=h2_full[:],
                    idxs_ap=pairsidx[:, q * 16 : (q + 1) * 16],
                    num_idxs=2 * P, num_idxs_reg=2 * P, elem_size=H,
                    transpose=True,
                )

                # xcat chunk ifc -> slice of xpT: drug (j, 0:128), target (j, 128:256)
                def xc(ifc):
                    j = ifc % 2
                    half = 0 if ifc < 2 else P
                    return xpT[:, j * 2 * P + half : j * 2 * P + half + P]

                yTr = wp.tile([P, MC * P], f16, name="yTr", tag="yTr", bufs=2)
                z = pp.tile([P, TILE2], f32, name="z", tag="msg0", bufs=2)
                for m in range(MC):
                    yT = pp.tile([P, 2 * TILE2], f32, name="yT", tag="aggT", bufs=4)
                    for k in range(KC):
                        nc.tensor.matmul(
                            yT[:, 0:P],
                            lhsT=fc1_sb[:, (k * MC + m) * P : (k * MC + m + 1) * P],
                            rhs=xc(k), start=(k == 0), stop=(k == KC - 1),
                        )
                    nc.scalar.activation(
                        yTr[:, m * P : (m + 1) * P], yT[:, 0:P],
                        mybir.ActivationFunctionType.Relu,
                        bias=fc1b_sb[:, m : m + 1], scale=1.0,
                    )
                    nc.tensor.matmul(
                        z[0:1, 0:P], lhsT=fc2_sb[:, m : m + 1],
                        rhs=yTr[:, m * P : (m + 1) * P],
                        start=(m == 0), stop=(m == MC - 1),
                    )
                zs = wp.tile([1, P], f32, name="zs", tag="zs", bufs=2)
                nc.scalar.activation(
                    zs[:], z[0:1, 0:P], mybir.ActivationFunctionType.Sigmoid,
                    bias=meta["fc2b"], scale=1.0,
                )
                nc.sync.dma_start(out_t.ap()[q * P : (q + 1) * P, :], zs[:])
    return nc


_NC_CACHE = []


def kernel(**inputs):
    from concourse import bass_utils

    meta, in_maps = _preprocess(inputs)
    key = (meta["N"], meta["H"], meta["R"], meta["TC"], meta["Q"],
           tuple(int(x) for x in meta["nch"]))
    if _NC_CACHE and _NC_CACHE[0][0] == key:
        nc = _NC_CACHE[0][1]
    else:
        nc = _build(meta)
        nc.compile()
        _NC_CACHE[:] = [(key, nc)]
    res = bass_utils.run_bass_kernel_spmd(nc, in_maps, core_ids=list(range(NCORES)))
    out = np.concatenate([res.results[c]["out"] for c in range(NCORES)], axis=0)
    return out.astype(np.float32)
